# revision 1
# baseline (speedup 1.0000x reference)
"""HGRN attention Trainium2 kernel.

Sharding: B*L (4 batches x 4096 tokens) split into 8 chunks of T=2048 tokens,
one per NeuronCore: core c = 2*b + half handles tokens [half*T, (half+1)*T) of
batch b. The gated linear recurrence h_t = sigmoid(f_t)*h_{t-1} + swiglu-input
runs per (batch, channel); the cross-chunk carry (h at the half boundary) is
exchanged with a tiny pairwise AllReduce and applied as h_local + cumprod*carry
(cumprod underflows to exactly 0 in fp32 past ~130 steps, so only the first 256
columns of each odd chunk need the fixup - bit-matching the fp32 reference).

On-chip layout is transposed ([channel, time]) so the recurrence maps onto the
DVE tensor_tensor_scan instruction; the host pre-transposes x and the weights,
and the kernel emits y transposed (host transposes back). Matmuls run in
float32r (tf32-rate, ~1.5e-4 rel err). RMSNorm over channels uses a full
ONES[128x128] matmul for the cross-partition reduce+broadcast.
"""
import numpy as np

import concourse.bacc as bacc
import concourse.tile as tile
import concourse.mybir as mybir
from concourse.bass_utils import run_bass_kernel_spmd

B, L, D = 4, 4096, 2048
T = 2048                 # tokens per core
NCORE = 8
ET = DT = D // 128       # 16 tiles of 128 channels
TB1 = 1024               # phase-1 time block
NB1 = T // TB1
TB2 = 512                # phase-2/3 time block
NB2 = T // TB2
CLEN = 256               # cumprod fixup length (0 in fp32 beyond this)
EPS = 1e-5

F32 = mybir.dt.float32
F32R = mybir.dt.float32r
AF = mybir.ActivationFunctionType
OP = mybir.AluOpType

_CACHE = {}


def _build():
    nc = bacc.Bacc("TRN2", target_bir_lowering=False, debug=False,
                   enable_asserts=True, num_devices=NCORE)
    xt_d = nc.dram_tensor("xt", [D, T], F32R, kind="ExternalInput")
    wi_d = nc.dram_tensor("wi", [D, D], F32R, kind="ExternalInput")   # (d_in, e)
    wf_d = nc.dram_tensor("wf", [D, D], F32R, kind="ExternalInput")
    wg_d = nc.dram_tensor("wg", [D, D], F32R, kind="ExternalInput")
    wo_d = nc.dram_tensor("wo", [D, D], F32R, kind="ExternalInput")   # (e, d_out)
    gnw_d = nc.dram_tensor("gnw", [128, ET], F32, kind="ExternalInput")
    mask_d = nc.dram_tensor("mask", [128, 1], F32, kind="ExternalInput")
    yt_d = nc.dram_tensor("yt", [D, T], F32, kind="ExternalOutput")

    with tile.TileContext(nc) as tc:
        with tc.tile_pool(name="persist", bufs=1) as pp, \
             tc.tile_pool(name="dram", bufs=1, space="DRAM") as dr:
            carry = pp.tile([128, ET], F32, tag="carry")
            recv = pp.tile([128, ET], F32, tag="recv")
            cin = pp.tile([128, ET], F32, tag="cin")
            gnw = pp.tile([128, ET], F32, tag="gnw")
            maskt = pp.tile([128, 1], F32, tag="mask")
            acc = pp.tile([128, T], F32, tag="acc")
            call = pp.tile([128, ET * CLEN], F32, tag="call")
            rms = pp.tile([128, T], F32, tag="rms")
            ones = pp.tile([128, 128], F32, tag="ones")

            h_sp = dr.tile([D, T], F32, tag="hsp")
            g_sp = dr.tile([D, T], F32, tag="gsp")
            hl_i = dr.tile([128, ET], F32, tag="hli")
            hl_o = dr.tile([128, ET], F32, tag="hlo")

            nc.vector.memset(carry[:], 0.0)
            nc.vector.memset(ones[:], 1.0)
            nc.sync.dma_start(gnw[:], gnw_d.ap()[:])
            nc.sync.dma_start(maskt[:], mask_d.ap()[:])

            # ---------------- phase 1: projections + scan + spills ----------
            with tc.tile_pool(name="xtp", bufs=1) as xtp, \
                 tc.tile_pool(name="wp", bufs=2) as wp, \
                 tc.tile_pool(name="wk", bufs=2) as wk, \
                 tc.tile_pool(name="pj", bufs=1, space="PSUM") as pj:
                for tb in range(NB1):
                    ts0 = tb * TB1
                    xt = xtp.tile([128, DT * TB1], F32R, tag="xt")
                    for dt in range(DT):
                        nc.sync.dma_start(
                            xt[:, dt * TB1:(dt + 1) * TB1],
                            xt_d.ap()[dt * 128:(dt + 1) * 128, ts0:ts0 + TB1])
                    nc.vector.memset(acc[:, ts0:ts0 + TB1], 0.0)
                    for et in range(ET):
                        es = slice(et * 128, (et + 1) * 128)
                        wts = []
                        for nm, wd in (("wi", wi_d), ("wf", wf_d), ("wg", wg_d)):
                            w = wp.tile([128, DT * 128], F32R, tag=nm)
                            nc.sync.dma_start(
                                w[:].rearrange("p (dt e) -> p dt e", e=128),
                                wd.ap().rearrange("(dt p) e -> p dt e",
                                                  p=128)[:, :, es])
                            wts.append(w)
                        ps = {}
                        for nm, w in zip(("i", "f", "g"), wts):
                            p = pj.tile([128, TB1], F32, tag="p" + nm)
                            for n in range(TB1 // 512):
                                for dt in range(DT):
                                    nc.tensor.matmul(
                                        p[:, n * 512:(n + 1) * 512],
                                        w[:, dt * 128:(dt + 1) * 128],
                                        xt[:, dt * TB1 + n * 512:
                                           dt * TB1 + (n + 1) * 512],
                                        start=(dt == 0), stop=(dt == DT - 1))
                            ps[nm] = p
                        gate = wk.tile([128, TB1], F32, tag="gate")
                        nc.scalar.activation(gate[:], ps["f"][:], AF.Sigmoid)
                        sil = wk.tile([128, TB1], F32, tag="sil")
                        nc.scalar.activation(sil[:], ps["i"][:], AF.Silu)
                        omg = wk.tile([128, TB1], F32, tag="omg")
                        nc.vector.tensor_scalar(omg[:], gate[:], -1.0, 1.0,
                                                OP.mult, OP.add)
                        iin = wk.tile([128, TB1], F32, tag="iin")
                        nc.vector.tensor_mul(iin[:], omg[:], sil[:])
                        h1 = wk.tile([128, TB1], F32, tag="h1")
                        nc.vector.tensor_tensor_scan(
                            h1[:], gate[:], iin[:], carry[:, et:et + 1],
                            OP.mult, OP.add)
                        nc.vector.tensor_copy(carry[:, et:et + 1],
                                              h1[:, TB1 - 1:TB1])
                        if tb == 0:
                            nc.vector.tensor_tensor_scan(
                                call[:, et * CLEN:(et + 1) * CLEN],
                                gate[:, 0:CLEN], gate[:, 0:CLEN], 1.0,
                                OP.mult, OP.bypass)
                        g1 = wk.tile([128, TB1], F32, tag="g1")
                        nc.scalar.copy(g1[:], ps["g"][:])
                        sq = wk.tile([128, TB1], F32, tag="sq")
                        nc.scalar.activation(sq[:], ps["g"][:], AF.Square)
                        nc.vector.tensor_add(acc[:, ts0:ts0 + TB1],
                                             acc[:, ts0:ts0 + TB1], sq[:])
                        nc.sync.dma_start(
                            h_sp[et * 128:(et + 1) * 128, ts0:ts0 + TB1], h1[:])
                        nc.sync.dma_start(
                            g_sp[et * 128:(et + 1) * 128, ts0:ts0 + TB1], g1[:])

            # ---------------- phase 1.5: carry exchange + rmsnorm -----------
            nc.sync.dma_start(hl_i[:], carry[:])
            nc.gpsimd.collective_compute(
                "AllReduce", OP.add,
                replica_groups=[[0, 1], [2, 3], [4, 5], [6, 7]],
                ins=[hl_i.opt()], outs=[hl_o.opt()])
            nc.sync.dma_start(recv[:], hl_o[:])
            nc.vector.tensor_sub(recv[:], recv[:], carry[:])
            nc.vector.tensor_scalar(cin[:], recv[:], maskt[:, 0:1], None,
                                    OP.mult)

            with tc.tile_pool(name="sp", bufs=1, space="PSUM") as sp, \
                 tc.tile_pool(name="rwk", bufs=1) as rwk:
                S = sp.tile([128, T], F32, tag="S")
                for n in range(T // 512):
                    nc.tensor.matmul(S[:, n * 512:(n + 1) * 512], ones[:],
                                     acc[:, n * 512:(n + 1) * 512],
                                     start=True, stop=True)
                m = rwk.tile([128, T], F32, tag="m")
                nc.vector.tensor_scalar(m[:], S[:], 1.0 / D, EPS,
                                        OP.mult, OP.add)
                rec = rwk.tile([128, T], F32, tag="rec")
                nc.vector.reciprocal(rec[:], m[:])
                nc.scalar.activation(rms[:], rec[:], AF.Sqrt)

            # ---------------- phase 2+3: gating + output projection ---------
            with tc.tile_pool(name="op2", bufs=2) as op2, \
                 tc.tile_pool(name="outp", bufs=2) as outp, \
                 tc.tile_pool(name="wop", bufs=2) as wop, \
                 tc.tile_pool(name="yp", bufs=2, space="PSUM") as yp, \
                 tc.tile_pool(name="yo", bufs=2) as yo:
                for tb2 in range(NB2):
                    ts = tb2 * TB2
                    osb = outp.tile([128, ET * TB2], F32R, tag="osb")
                    for et in range(ET):
                        h2 = op2.tile([128, TB2], F32, tag="h2")
                        nc.sync.dma_start(
                            h2[:], h_sp[et * 128:(et + 1) * 128, ts:ts + TB2])
                        g2 = op2.tile([128, TB2], F32, tag="g2")
                        nc.sync.dma_start(
                            g2[:], g_sp[et * 128:(et + 1) * 128, ts:ts + TB2])
                        if tb2 == 0:
                            nc.vector.scalar_tensor_tensor(
                                h2[:, 0:CLEN],
                                call[:, et * CLEN:(et + 1) * CLEN],
                                cin[:, et:et + 1], h2[:, 0:CLEN],
                                OP.mult, OP.add)
                        sw = op2.tile([128, TB2], F32, tag="sw")
                        nc.scalar.activation(sw[:], h2[:], AF.Silu)
                        w1 = op2.tile([128, TB2], F32, tag="w1")
                        nc.vector.tensor_mul(w1[:], g2[:], rms[:, ts:ts + TB2])
                        nc.vector.scalar_tensor_tensor(
                            osb[:, et * TB2:(et + 1) * TB2], w1[:],
                            gnw[:, et:et + 1], sw[:], OP.mult, OP.mult)
                    for dt in range(DT):
                        wo = wop.tile([128, ET * 128], F32R, tag="wo")
                        nc.sync.dma_start(
                            wo[:].rearrange("p (et d) -> p et d", d=128),
                            wo_d.ap().rearrange("(et p) d -> p et d",
                                                p=128)[:, :, dt * 128:(dt + 1) * 128])
                        ypt = yp.tile([128, TB2], F32, tag="ypt")
                        for et in range(ET):
                            nc.tensor.matmul(
                                ypt[:], wo[:, et * 128:(et + 1) * 128],
                                osb[:, et * TB2:(et + 1) * TB2],
                                start=(et == 0), stop=(et == ET - 1))
                        ysb = yo.tile([128, TB2], F32, tag="ysb")
                        nc.scalar.copy(ysb[:], ypt[:])
                        nc.sync.dma_start(
                            yt_d.ap()[dt * 128:(dt + 1) * 128, ts:ts + TB2],
                            ysb[:])
    nc.compile()
    return nc


def _get_nc():
    if "nc" not in _CACHE:
        _CACHE["nc"] = _build()
    return _CACHE["nc"]


def kernel(hidden_states, Wi, Wf, Wg, g_norm_weight, Wo, **_unused):
    nc = _get_nc()
    wiT = np.ascontiguousarray(Wi.T)
    wfT = np.ascontiguousarray(Wf.T)
    wgT = np.ascontiguousarray(Wg.T)
    woT = np.ascontiguousarray(Wo.T)
    gnw = np.ascontiguousarray(
        np.asarray(g_norm_weight, np.float32).reshape(ET, 128).T)
    in_maps = []
    for c in range(NCORE):
        b, half = c // 2, c % 2
        xt = np.ascontiguousarray(
            hidden_states[b, half * T:(half + 1) * T, :].T)
        mask = np.full((128, 1), float(half), np.float32)
        in_maps.append({"xt": xt, "wi": wiT, "wf": wfT, "wg": wgT,
                        "wo": woT, "gnw": gnw, "mask": mask})
    res = run_bass_kernel_spmd(nc, in_maps, list(range(NCORE))).results
    y = np.empty((B, L, D), np.float32)
    for c in range(NCORE):
        b, half = c // 2, c % 2
        y[b, half * T:(half + 1) * T, :] = res[c]["yt"].T
    return y



# revision 2
# speedup vs baseline: 1.4412x; 1.4412x over previous
"""HGRN attention Trainium2 kernel (v2).

Sharding: B*L (4 batches x 4096 tokens) split into 8 chunks of T=2048 tokens,
one per NeuronCore: core c = 2*b + half handles tokens [half*T, (half+1)*T) of
batch b. The gated linear recurrence h_t = sigmoid(f_t)*h_{t-1} + swiglu-input
runs per (batch, channel); the cross-chunk carry (h at the half boundary) is
exchanged with a tiny pairwise AllReduce and applied as h_local + cumprod*carry
(cumprod underflows to 0 in fp32 past ~130 steps, so only the first 256
columns of each odd chunk need the fixup).

v2 layout (vs v1): x is kept SBUF-resident in bf16 for the whole of phase 1
and the et loop runs over the full T per weight tile, so Wi/Wf/Wg are loaded
exactly once (host pre-tiles them in bf16). h and g spill to DRAM in bf16.
Phase 2 keeps all Wo tiles resident (bf16), assembles osb per 512-column time
chunk and streams the o-projection; chunk 0 (the only one that needs the
cross-core carry fixup) is processed LAST so the AllReduce hides under the
other chunks' compute. rms is computed per-chunk inside the last et iteration.
Matmul moving operands are bf16 (1 cycle/row); end-to-end rel err ~5e-3.
"""
import numpy as np
import ml_dtypes

import concourse.bacc as bacc
import concourse.tile as tile
import concourse.mybir as mybir
from concourse.bass_utils import run_bass_kernel_spmd

B, L, D = 4, 4096, 2048
T = 2048                 # tokens per core
NCORE = 8
ET = DT = D // 128       # 16 tiles of 128 channels
CH = 512                 # time chunk (one PSUM bank)
NC = T // CH             # 4
CLEN = 256               # cumprod fixup length (0 in fp32 beyond this)
EPS = 1e-5

F32 = mybir.dt.float32
BF16 = mybir.dt.bfloat16
AF = mybir.ActivationFunctionType
OP = mybir.AluOpType

_CACHE = {}


def _build():
    nc = bacc.Bacc("TRN2", target_bir_lowering=False, debug=False,
                   enable_asserts=True, num_devices=NCORE)
    xt_d = nc.dram_tensor("xt", [D, T], BF16, kind="ExternalInput")
    # host-pre-tiled weights: row block et covers lhsT tiles [128, DT*128]
    wi_d = nc.dram_tensor("wi", [ET * 128, DT * 128], BF16, kind="ExternalInput")
    wf_d = nc.dram_tensor("wf", [ET * 128, DT * 128], BF16, kind="ExternalInput")
    wg_d = nc.dram_tensor("wg", [ET * 128, DT * 128], BF16, kind="ExternalInput")
    wo_d = nc.dram_tensor("wo", [DT * 128, ET * 128], BF16, kind="ExternalInput")
    gnw_d = nc.dram_tensor("gnw", [128, ET], F32, kind="ExternalInput")
    mask_d = nc.dram_tensor("mask", [128, 1], F32, kind="ExternalInput")
    yt_d = nc.dram_tensor("yt", [D, T], BF16, kind="ExternalOutput")

    with tile.TileContext(nc) as tc:
        with tc.tile_pool(name="persist", bufs=1) as pp, \
             tc.tile_pool(name="dram", bufs=1, space="DRAM") as dr:
            carry = pp.tile([128, ET], F32, tag="carry")
            recv = pp.tile([128, ET], F32, tag="recv")
            cin = pp.tile([128, ET], F32, tag="cin")
            gnw = pp.tile([128, ET], F32, tag="gnw")
            maskt = pp.tile([128, 1], F32, tag="mask")
            call = pp.tile([128, ET * CLEN], F32, tag="call")
            acc = pp.tile([128, T], F32, tag="acc")
            rms = pp.tile([128, T], F32, tag="rms")
            onesb = pp.tile([128, 128], BF16, tag="ones")

            h_sp = dr.tile([D, T], BF16, tag="hsp")
            g_sp = dr.tile([D, T], BF16, tag="gsp")
            hl_i = dr.tile([128, ET], F32, tag="hli")
            hl_o = dr.tile([128, ET], F32, tag="hlo")

            nc.vector.memset(carry[:], 0.0)
            nc.vector.memset(onesb[:], 1.0)
            nc.vector.memset(acc[:], 0.0)
            nc.sync.dma_start(gnw[:], gnw_d.ap()[:])
            nc.sync.dma_start(maskt[:], mask_d.ap()[:])

            # ---------------- phase 1: projections + scan + bf16 spills -----
            with tc.tile_pool(name="xp", bufs=1) as xp, \
                 tc.tile_pool(name="wp", bufs=2) as wp, \
                 tc.tile_pool(name="hg", bufs=2) as hgp, \
                 tc.tile_pool(name="wk", bufs=2) as wk, \
                 tc.tile_pool(name="pj", bufs=2, space="PSUM") as pj, \
                 tc.tile_pool(name="prms", bufs=2, space="PSUM") as prms:
                x_sb = xp.tile([128, DT * T], BF16, tag="x")
                xv = x_sb[:].rearrange("p (dt t) -> p dt t", t=T)
                xs_d = xt_d.ap().rearrange("(dt p) t -> p dt t", p=128)
                # chunk 0 first so the first matmuls start ASAP
                nc.sync.dma_start(xv[:, :, 0:CH], xs_d[:, :, 0:CH])
                for et in range(ET):
                    ws = []
                    for nm, wd in (("wf", wf_d), ("wi", wi_d), ("wg", wg_d)):
                        w = wp.tile([128, DT * 128], BF16, tag=nm)
                        nc.sync.dma_start(w[:], wd.ap()[et * 128:(et + 1) * 128, :])
                        ws.append(w)
                    wfv, wiv, wgv = ws
                    if et == 0:
                        for c in range(1, NC):
                            nc.sync.dma_start(xv[:, :, c * CH:(c + 1) * CH],
                                              xs_d[:, :, c * CH:(c + 1) * CH])
                    h_et = hgp.tile([128, T], BF16, tag="h")
                    g_et = hgp.tile([128, T], BF16, tag="g")
                    for c in range(NC):
                        cs = slice(c * CH, (c + 1) * CH)
                        pf = pj.tile([128, CH], F32, tag="pf")
                        pi = pj.tile([128, CH], F32, tag="pi")
                        pg = pj.tile([128, CH], F32, tag="pg")
                        for ps, w in ((pf, wfv), (pi, wiv), (pg, wgv)):
                            for dt in range(DT):
                                nc.tensor.matmul(
                                    ps[:], w[:, dt * 128:(dt + 1) * 128],
                                    x_sb[:, dt * T + c * CH:dt * T + (c + 1) * CH],
                                    start=(dt == 0), stop=(dt == DT - 1))
                        gate = wk.tile([128, CH], F32, tag="gate")
                        nc.scalar.activation(gate[:], pf[:], AF.Sigmoid)
                        sil = wk.tile([128, CH], F32, tag="sil")
                        nc.scalar.activation(sil[:], pi[:], AF.Silu)
                        omg = wk.tile([128, CH], F32, tag="omg")
                        nc.vector.tensor_scalar(omg[:], gate[:], -1.0, 1.0,
                                                OP.mult, OP.add)
                        iin = wk.tile([128, CH], F32, tag="iin")
                        nc.vector.tensor_mul(iin[:], omg[:], sil[:])
                        nc.vector.tensor_tensor_scan(
                            h_et[:, cs], gate[:], iin[:], carry[:, et:et + 1],
                            OP.mult, OP.add)
                        nc.vector.tensor_copy(
                            carry[:, et:et + 1],
                            h_et[:, c * CH + CH - 1:c * CH + CH])
                        if c == 0:
                            nc.vector.tensor_tensor_scan(
                                call[:, et * CLEN:(et + 1) * CLEN],
                                gate[:, 0:CLEN], gate[:, 0:CLEN], 1.0,
                                OP.mult, OP.bypass)
                        nc.scalar.copy(g_et[:, cs], pg[:])
                        sq = wk.tile([128, CH], F32, tag="sq")
                        nc.scalar.activation(sq[:], pg[:], AF.Square)
                        nc.vector.tensor_add(acc[:, cs], acc[:, cs], sq[:])
                        if et == ET - 1:
                            # rms for this time chunk (acc now complete)
                            accb = wk.tile([128, CH], BF16, tag="accb")
                            nc.scalar.copy(accb[:], acc[:, cs])
                            S = prms.tile([128, CH], F32, tag="S")
                            nc.tensor.matmul(S[:], onesb[:], accb[:],
                                             start=True, stop=True)
                            m = wk.tile([128, CH], F32, tag="m")
                            nc.vector.tensor_scalar(m[:], S[:], 1.0 / D, EPS,
                                                    OP.mult, OP.add)
                            rec = wk.tile([128, CH], F32, tag="rec")
                            nc.vector.reciprocal(rec[:], m[:])
                            nc.scalar.activation(rms[:, cs], rec[:], AF.Sqrt)
                    nc.sync.dma_start(h_sp[et * 128:(et + 1) * 128, :], h_et[:])
                    nc.sync.dma_start(g_sp[et * 128:(et + 1) * 128, :], g_et[:])

            # ---------------- phase 1.5: carry exchange ---------------------
            nc.sync.dma_start(hl_i[:], carry[:])
            nc.gpsimd.collective_compute(
                "AllReduce", OP.add,
                replica_groups=[[0, 1], [2, 3], [4, 5], [6, 7]],
                ins=[hl_i.opt()], outs=[hl_o.opt()])
            nc.sync.dma_start(recv[:], hl_o[:])
            nc.vector.tensor_sub(recv[:], recv[:], carry[:])
            nc.vector.tensor_scalar(cin[:], recv[:], maskt[:, 0:1], None,
                                    OP.mult)

            # ---------------- phase 2: gating + output projection -----------
            # chunk 0 (needs the carry fixup) last, so the collective hides
            with tc.tile_pool(name="wop", bufs=1) as wop, \
                 tc.tile_pool(name="osp", bufs=2) as osp, \
                 tc.tile_pool(name="hgl", bufs=6) as hgl, \
                 tc.tile_pool(name="w2", bufs=3) as w2, \
                 tc.tile_pool(name="ycp", bufs=2) as ycp, \
                 tc.tile_pool(name="yp", bufs=2, space="PSUM") as yp:
                wos = []
                for dt in range(DT):
                    wo = wop.tile([128, ET * 128], BF16, tag=f"wo{dt}")
                    nc.sync.dma_start(wo[:], wo_d.ap()[dt * 128:(dt + 1) * 128, :])
                    wos.append(wo)
                for c in (1, 2, 3, 0):
                    cs = slice(c * CH, (c + 1) * CH)
                    osb = osp.tile([128, ET * CH], BF16, tag="osb")
                    for et in range(ET):
                        hc = hgl.tile([128, CH], BF16, tag="hc")
                        nc.sync.dma_start(hc[:],
                                          h_sp[et * 128:(et + 1) * 128, cs])
                        gc = hgl.tile([128, CH], BF16, tag="gc")
                        nc.sync.dma_start(gc[:],
                                          g_sp[et * 128:(et + 1) * 128, cs])
                        sw = w2.tile([128, CH], F32, tag="sw")
                        if c == 0:
                            hf = w2.tile([128, CH], F32, tag="hf")
                            nc.scalar.copy(hf[:], hc[:])
                            nc.vector.scalar_tensor_tensor(
                                hf[:, 0:CLEN],
                                call[:, et * CLEN:(et + 1) * CLEN],
                                cin[:, et:et + 1], hf[:, 0:CLEN],
                                OP.mult, OP.add)
                            nc.scalar.activation(sw[:], hf[:], AF.Silu)
                        else:
                            nc.scalar.activation(sw[:], hc[:], AF.Silu)
                        gf = w2.tile([128, CH], F32, tag="gf")
                        nc.scalar.copy(gf[:], gc[:])
                        w1 = w2.tile([128, CH], F32, tag="w1")
                        nc.vector.tensor_mul(w1[:], gf[:], rms[:, cs])
                        nc.vector.scalar_tensor_tensor(
                            osb[:, et * CH:(et + 1) * CH], w1[:],
                            gnw[:, et:et + 1], sw[:], OP.mult, OP.mult)
                    ych = ycp.tile([128, DT * CH], BF16, tag="ych")
                    for dt in range(DT):
                        ypt = yp.tile([128, CH], F32, tag="ypt")
                        for et in range(ET):
                            nc.tensor.matmul(
                                ypt[:], wos[dt][:, et * 128:(et + 1) * 128],
                                osb[:, et * CH:(et + 1) * CH],
                                start=(et == 0), stop=(et == ET - 1))
                        nc.scalar.copy(ych[:, dt * CH:(dt + 1) * CH], ypt[:])
                    nc.sync.dma_start(
                        yt_d.ap().rearrange("(dt p) t -> p dt t", p=128)[:, :, cs],
                        ych[:].rearrange("p (dt t) -> p dt t", t=CH))
    nc.compile()
    return nc


def _get_nc():
    if "nc" not in _CACHE:
        _CACHE["nc"] = _build()
    return _CACHE["nc"]


def _tile_lhs(WT):
    """[K*128, M*128] host mat -> pre-tiled [M_blk*128, K_blk*128] so row block
    m is the lhsT tile [128, K*128] with columns grouped by contraction block."""
    KB = WT.shape[0] // 128
    MB = WT.shape[1] // 128
    t = WT.reshape(KB, 128, MB, 128).transpose(2, 1, 0, 3)
    return np.ascontiguousarray(t.reshape(MB * 128, KB * 128)
                                .astype(ml_dtypes.bfloat16))


def kernel(hidden_states, Wi, Wf, Wg, g_norm_weight, Wo, **_unused):
    nc = _get_nc()
    bf = ml_dtypes.bfloat16
    wiT = _tile_lhs(np.ascontiguousarray(Wi.T))
    wfT = _tile_lhs(np.ascontiguousarray(Wf.T))
    wgT = _tile_lhs(np.ascontiguousarray(Wg.T))
    woT = _tile_lhs(np.ascontiguousarray(Wo.T))
    gnw = np.ascontiguousarray(
        np.asarray(g_norm_weight, np.float32).reshape(ET, 128).T)
    in_maps = []
    for c in range(NCORE):
        b, half = c // 2, c % 2
        xt = np.ascontiguousarray(
            hidden_states[b, half * T:(half + 1) * T, :].T.astype(bf))
        mask = np.full((128, 1), float(half), np.float32)
        in_maps.append({"xt": xt, "wi": wiT, "wf": wfT, "wg": wgT,
                        "wo": woT, "gnw": gnw, "mask": mask})
    _CACHE["in_maps"] = in_maps
    res = run_bass_kernel_spmd(nc, in_maps, list(range(NCORE))).results
    y = np.empty((B, L, D), np.float32)
    for c in range(NCORE):
        b, half = c // 2, c % 2
        y[b, half * T:(half + 1) * T, :] = res[c]["yt"].astype(np.float32).T
    return y


# revision 3
# speedup vs baseline: 1.4899x; 1.0338x over previous
"""HGRN attention Trainium2 kernel (v3).

Sharding: B*L (4 batches x 4096 tokens) split into 8 chunks of T=2048 tokens,
one per NeuronCore: core c = 2*b + half handles tokens [half*T, (half+1)*T) of
batch b. The gated linear recurrence h_t = sigmoid(f_t)*h_{t-1} + swiglu-input
runs per (batch, channel); the cross-chunk carry (h at the half boundary) is
exchanged with a tiny pairwise AllReduce and applied as h_local + cumprod*carry
(cumprod underflows to 0 in fp32 past ~130 steps, so only the first 256
columns of each odd chunk need the fixup).

v3 structure: phase 1 keeps x SBUF-resident (bf16), runs the et loop over the
full T per weight tile (Wi/Wf/Wg loaded exactly once, host pre-tiled bf16),
and — because rms is a per-time-column scalar that commutes through the
o-projection — computes the full gate product osb' = g*gnw*silu(h) inline
(g straight from PSUM), spilling only osb' in bf16 (last two et tiles stay
in SBUF). Phase 2 is pure o-projection matmul streaming; rms multiplies the
PSUM result on the way out (DVE). Chunk 0 (the only one needing the carry
fixup, via saved h/g prefixes) is processed last so the AllReduce latency
hides; all loads are prefetched ahead of the y stores to avoid queue
head-of-line blocking. End-to-end rel err ~5e-3 (vs 2e-2 gate).
"""
import numpy as np
import ml_dtypes

import concourse.bacc as bacc
import concourse.tile as tile
import concourse.mybir as mybir
from concourse.bass_utils import run_bass_kernel_spmd

B, L, D = 4, 4096, 2048
T = 2048                 # tokens per core
NCORE = 8
ET = DT = D // 128       # 16 tiles of 128 channels
CH = 512                 # time chunk (one PSUM bank)
NC = T // CH             # 4
CLEN = 256               # cumprod fixup length (0 in fp32 beyond this)
ETS = ET - 2             # et tiles spilled to DRAM (last 2 stay in SBUF)
EPS = 1e-5

F32 = mybir.dt.float32
BF16 = mybir.dt.bfloat16
AF = mybir.ActivationFunctionType
OP = mybir.AluOpType

_CACHE = {}


def _build():
    nc = bacc.Bacc("TRN2", target_bir_lowering=False, debug=False,
                   enable_asserts=True, num_devices=NCORE)
    xt_d = nc.dram_tensor("xt", [D, T], BF16, kind="ExternalInput")
    # host-pre-tiled weights: row block et is the lhsT tile [128, DT*128]
    wi_d = nc.dram_tensor("wi", [ET * 128, DT * 128], BF16, kind="ExternalInput")
    wf_d = nc.dram_tensor("wf", [ET * 128, DT * 128], BF16, kind="ExternalInput")
    wg_d = nc.dram_tensor("wg", [ET * 128, DT * 128], BF16, kind="ExternalInput")
    wo_d = nc.dram_tensor("wo", [DT * 128, ET * 128], BF16, kind="ExternalInput")
    gnw_d = nc.dram_tensor("gnw", [128, ET], F32, kind="ExternalInput")
    mask_d = nc.dram_tensor("mask", [128, 1], F32, kind="ExternalInput")
    yt_d = nc.dram_tensor("yt", [D, T], BF16, kind="ExternalOutput")

    with tile.TileContext(nc) as tc:
        with tc.tile_pool(name="persist", bufs=1) as pp, \
             tc.tile_pool(name="dram", bufs=1, space="DRAM") as dr, \
             tc.tile_pool(name="hg", bufs=2) as hgp:
            carry = pp.tile([128, ET], F32, tag="carry")
            recv = pp.tile([128, ET], F32, tag="recv")
            cin = pp.tile([128, ET], F32, tag="cin")
            gnw = pp.tile([128, ET], F32, tag="gnw")
            maskt = pp.tile([128, 1], F32, tag="mask")
            call = pp.tile([128, ET * CLEN], F32, tag="call")
            h0sb = pp.tile([128, ET * CLEN], BF16, tag="h0")
            g0sb = pp.tile([128, ET * CLEN], BF16, tag="g0")
            acc = pp.tile([128, T], F32, tag="acc")
            rms = pp.tile([128, T], F32, tag="rms")
            onesb = pp.tile([128, 128], BF16, tag="ones")

            osb_sp = dr.tile([D, T], BF16, tag="osp")
            hl_i = dr.tile([128, ET], F32, tag="hli")
            hl_o = dr.tile([128, ET], F32, tag="hlo")

            nc.vector.memset(carry[:], 0.0)
            nc.vector.memset(onesb[:], 1.0)
            nc.vector.memset(acc[:], 0.0)
            nc.sync.dma_start(gnw[:], gnw_d.ap()[:])
            nc.sync.dma_start(maskt[:], mask_d.ap()[:])

            osb_live = {}   # et -> SBUF tile for the unspilled tail ets

            # ---------------- phase 1: proj + scan + fused gating -----------
            with tc.tile_pool(name="xp", bufs=1) as xp, \
                 tc.tile_pool(name="wp", bufs=2) as wp, \
                 tc.tile_pool(name="wk", bufs=2) as wk, \
                 tc.tile_pool(name="pj", bufs=2, space="PSUM") as pj, \
                 tc.tile_pool(name="prms", bufs=2, space="PSUM") as prms:
                x_sb = xp.tile([128, DT * T], BF16, tag="x")
                xv = x_sb[:].rearrange("p (dt t) -> p dt t", t=T)
                xs_d = xt_d.ap().rearrange("(dt p) t -> p dt t", p=128)
                for et in range(ET):
                    ws = []
                    for nm, wd in (("wf", wf_d), ("wi", wi_d), ("wg", wg_d)):
                        w = wp.tile([128, DT * 128], BF16, tag=nm)
                        nc.sync.dma_start(w[:], wd.ap()[et * 128:(et + 1) * 128, :])
                        ws.append(w)
                        if et == 0 and nm == "wf":
                            # x chunk 0 right after the first weight tile
                            nc.sync.dma_start(xv[:, :, 0:CH], xs_d[:, :, 0:CH])
                    wfv, wiv, wgv = ws
                    if et == 0:
                        for c in range(1, NC):
                            nc.sync.dma_start(xv[:, :, c * CH:(c + 1) * CH],
                                              xs_d[:, :, c * CH:(c + 1) * CH])
                    h_et = hgp.tile([128, T], BF16, tag="h")
                    o_et = hgp.tile([128, T], BF16, tag="osb")
                    for c in range(NC):
                        cs = slice(c * CH, (c + 1) * CH)
                        pf = pj.tile([128, CH], F32, tag="pf")
                        pi = pj.tile([128, CH], F32, tag="pi")
                        pg = pj.tile([128, CH], F32, tag="pg")
                        for ps, w in ((pf, wfv), (pi, wiv), (pg, wgv)):
                            for dt in range(DT):
                                nc.tensor.matmul(
                                    ps[:], w[:, dt * 128:(dt + 1) * 128],
                                    x_sb[:, dt * T + c * CH:dt * T + (c + 1) * CH],
                                    start=(dt == 0), stop=(dt == DT - 1))
                        gate = wk.tile([128, CH], F32, tag="gate")
                        nc.scalar.activation(gate[:], pf[:], AF.Sigmoid)
                        sil = wk.tile([128, CH], F32, tag="sil")
                        nc.scalar.activation(sil[:], pi[:], AF.Silu)
                        omg = wk.tile([128, CH], F32, tag="omg")
                        nc.vector.tensor_scalar(omg[:], gate[:], -1.0, 1.0,
                                                OP.mult, OP.add)
                        iin = wk.tile([128, CH], F32, tag="iin")
                        nc.vector.tensor_mul(iin[:], omg[:], sil[:])
                        nc.vector.tensor_tensor_scan(
                            h_et[:, cs], gate[:], iin[:], carry[:, et:et + 1],
                            OP.mult, OP.add)
                        nc.vector.tensor_copy(
                            carry[:, et:et + 1],
                            h_et[:, c * CH + CH - 1:c * CH + CH])
                        if c == 0:
                            nc.vector.tensor_tensor_scan(
                                call[:, et * CLEN:(et + 1) * CLEN],
                                gate[:, 0:CLEN], gate[:, 0:CLEN], 1.0,
                                OP.mult, OP.bypass)
                            nc.vector.tensor_copy(
                                h0sb[:, et * CLEN:(et + 1) * CLEN],
                                h_et[:, 0:CLEN])
                            nc.scalar.copy(
                                g0sb[:, et * CLEN:(et + 1) * CLEN],
                                pg[:, 0:CLEN])
                        swh = wk.tile([128, CH], F32, tag="swh")
                        nc.scalar.activation(swh[:], h_et[:, cs], AF.Silu)
                        nc.vector.scalar_tensor_tensor(
                            o_et[:, cs], pg[:], gnw[:, et:et + 1], swh[:],
                            OP.mult, OP.mult)
                        sq = wk.tile([128, CH], F32, tag="sq")
                        nc.scalar.activation(sq[:], pg[:], AF.Square)
                        nc.vector.tensor_add(acc[:, cs], acc[:, cs], sq[:])
                        if et == ET - 1:
                            # rms for this time chunk (acc now complete)
                            accb = wk.tile([128, CH], BF16, tag="accb")
                            nc.scalar.copy(accb[:], acc[:, cs])
                            S = prms.tile([128, CH], F32, tag="S")
                            nc.tensor.matmul(S[:], onesb[:], accb[:],
                                             start=True, stop=True)
                            m = wk.tile([128, CH], F32, tag="m")
                            nc.vector.tensor_scalar(m[:], S[:], 1.0 / D, EPS,
                                                    OP.mult, OP.add)
                            rec = wk.tile([128, CH], F32, tag="rec")
                            nc.vector.reciprocal(rec[:], m[:])
                            nc.scalar.activation(rms[:, cs], rec[:], AF.Sqrt)
                    if et < ETS:
                        nc.sync.dma_start(osb_sp[et * 128:(et + 1) * 128, :],
                                          o_et[:])
                    else:
                        osb_live[et] = o_et

            # ---------------- phase 1.5: carry exchange ---------------------
            nc.sync.dma_start(hl_i[:], carry[:])
            nc.gpsimd.collective_compute(
                "AllReduce", OP.add,
                replica_groups=[[0, 1], [2, 3], [4, 5], [6, 7]],
                ins=[hl_i.opt()], outs=[hl_o.opt()])

            # ---------------- phase 2: output projection --------------------
            # chunk 0 (needs the carry fixup) last, so the collective hides
            corder = (1, 2, 3, 0)
            osd = osb_sp[0:ETS * 128, :].rearrange("(et p) t -> p et t", p=128)
            with tc.tile_pool(name="wop", bufs=1) as wop, \
                 tc.tile_pool(name="oscp", bufs=3) as oscp, \
                 tc.tile_pool(name="w2", bufs=3) as w2, \
                 tc.tile_pool(name="ycp", bufs=3) as ycp, \
                 tc.tile_pool(name="yp", bufs=2, space="PSUM") as yp:
                oscs = {}

                def load_osc(c):
                    t = oscp.tile([128, ETS * CH], BF16, tag="osc")
                    nc.sync.dma_start(
                        t[:].rearrange("p (et t) -> p et t", t=CH),
                        osd[:, :, c * CH:(c + 1) * CH])
                    oscs[c] = t

                load_osc(corder[0])
                wos = []
                for dt in range(DT):
                    wo = wop.tile([128, ET * 128], BF16, tag=f"wo{dt}")
                    nc.sync.dma_start(wo[:], wo_d.ap()[dt * 128:(dt + 1) * 128, :])
                    wos.append(wo)
                yv = yt_d.ap().rearrange("(dt p) t -> p dt t", p=128)
                for ci, c in enumerate(corder):
                    cs = slice(c * CH, (c + 1) * CH)
                    if ci + 1 < NC:
                        load_osc(corder[ci + 1])
                    osc = oscs.pop(c)
                    if c == 0:
                        # collective result -> cin, then fix osb chunk-0 prefix
                        nc.sync.dma_start(recv[:], hl_o[:])
                        nc.vector.tensor_sub(recv[:], recv[:], carry[:])
                        nc.vector.tensor_scalar(cin[:], recv[:],
                                                maskt[:, 0:1], None, OP.mult)
                        for et in range(ET):
                            es = slice(et * CLEN, (et + 1) * CLEN)
                            hf = w2.tile([128, CLEN], F32, tag="hf")
                            nc.scalar.copy(hf[:], h0sb[:, es])
                            nc.vector.scalar_tensor_tensor(
                                hf[:], call[:, es], cin[:, et:et + 1], hf[:],
                                OP.mult, OP.add)
                            swf = w2.tile([128, CLEN], F32, tag="swf")
                            nc.scalar.activation(swf[:], hf[:], AF.Silu)
                            gf0 = w2.tile([128, CLEN], F32, tag="gf0")
                            nc.scalar.copy(gf0[:], g0sb[:, es])
                            dst = (osc[:, et * CH:et * CH + CLEN] if et < ETS
                                   else osb_live[et][:, 0:CLEN])
                            nc.vector.scalar_tensor_tensor(
                                dst, gf0[:], gnw[:, et:et + 1], swf[:],
                                OP.mult, OP.mult)
                    for dg in range(4):          # groups of 4 dt tiles
                        ych = ycp.tile([128, 4 * CH], BF16, tag="ych")
                        for dl in range(4):
                            dt = dg * 4 + dl
                            ypt = yp.tile([128, CH], F32, tag="ypt")
                            for et in range(ET):
                                src = (osc[:, et * CH:(et + 1) * CH] if et < ETS
                                       else osb_live[et][:, cs])
                                nc.tensor.matmul(
                                    ypt[:], wos[dt][:, et * 128:(et + 1) * 128],
                                    src, start=(et == 0), stop=(et == ET - 1))
                            nc.vector.tensor_mul(ych[:, dl * CH:(dl + 1) * CH],
                                                 ypt[:], rms[:, cs])
                        nc.sync.dma_start(
                            yv[:, dg * 4:(dg + 1) * 4, cs],
                            ych[:].rearrange("p (dt t) -> p dt t", t=CH))
    nc.compile()
    return nc


def _get_nc():
    if "nc" not in _CACHE:
        _CACHE["nc"] = _build()
    return _CACHE["nc"]


def _tile_lhs(WT):
    """[K*128, M*128] host mat -> pre-tiled [M_blk*128, K_blk*128] so row block
    m is the lhsT tile [128, K*128] with columns grouped by contraction block."""
    KB = WT.shape[0] // 128
    MB = WT.shape[1] // 128
    t = WT.reshape(KB, 128, MB, 128).transpose(2, 1, 0, 3)
    return np.ascontiguousarray(t.reshape(MB * 128, KB * 128)
                                .astype(ml_dtypes.bfloat16))


def kernel(hidden_states, Wi, Wf, Wg, g_norm_weight, Wo, **_unused):
    nc = _get_nc()
    bf = ml_dtypes.bfloat16
    wiT = _tile_lhs(np.ascontiguousarray(Wi.T))
    wfT = _tile_lhs(np.ascontiguousarray(Wf.T))
    wgT = _tile_lhs(np.ascontiguousarray(Wg.T))
    woT = _tile_lhs(np.ascontiguousarray(Wo.T))
    gnw = np.ascontiguousarray(
        np.asarray(g_norm_weight, np.float32).reshape(ET, 128).T)
    in_maps = []
    for c in range(NCORE):
        b, half = c // 2, c % 2
        xt = np.ascontiguousarray(
            hidden_states[b, half * T:(half + 1) * T, :].T.astype(bf))
        mask = np.full((128, 1), float(half), np.float32)
        in_maps.append({"xt": xt, "wi": wiT, "wf": wfT, "wg": wgT,
                        "wo": woT, "gnw": gnw, "mask": mask})
    _CACHE["in_maps"] = in_maps
    res = run_bass_kernel_spmd(nc, in_maps, list(range(NCORE))).results
    y = np.empty((B, L, D), np.float32)
    for c in range(NCORE):
        b, half = c // 2, c % 2
        y[b, half * T:(half + 1) * T, :] = res[c]["yt"].astype(np.float32).T
    return y
